# revision 38
# baseline (speedup 1.0000x reference)
"""MeshAttention kernel for 8x Trainium2 NeuronCores.

Problem (per batch element b, one per core):
  idx   = knn16(xyz[b])                      # 16-NN incl. self by squared dist
  q     = concat[b] @ Wq.T + bq
  u     = (Wk.T Wq) concat[b] + Wk.T bq      # scores: s[n,m] = u_n . c_m
  score[n,k] = (q_n . key) ... == u_n . c_idx + const_n  (const drops in softmax)
  attn  = softmax over the 16 selected m of s[n,m]/sqrt(D)
  out[b] = (attn-weighted sum of feat rows).T  # [C, N]

Device strategy (dense, gather-free):
  per 128-row tile: PE computes pd'[n,m] = 2 x_n.x_m - |x_m|^2 (row-rank
  equivalent to the reference's squared-distance matrix) in PSUM; DVE max8
  extracts per-512-chunk top-8 candidates and then the 16th-largest value t;
  ACT writes pen = relu(t - pd) to SBUF (exact 0 on the selected 16); PE
  accumulates s = (u.c)/sqrt(D) and injects -lam*pen via an identity matmul;
  ACT takes exp with accum_out giving the softmax denominator Z; HWDGE DMA
  transposes the unnormalized attention to m-major chunks; PE contracts with
  the feature table giving out^T unnormalized. Host divides by Z.
"""

import sys

sys.path.insert(0, "/opt/trn_rl_repo")

import numpy as np
import ml_dtypes

import jax

# Persistent compile cache: the walrus/birsim compile of this program takes
# ~5 min; cache the PJRT executable so repeat runs (and fresh processes on
# the same machine) skip it.
try:
    jax.config.update("jax_compilation_cache_dir", "/tmp/jax_neff_cache")
    jax.config.update("jax_persistent_cache_min_compile_time_secs", 1.0)
    jax.config.update("jax_persistent_cache_min_entry_size_bytes", 0)
except Exception:
    pass

import concourse.bass as bass
import concourse.bacc as bacc
import concourse.mybir as mybir
from concourse import tile
from concourse.bass_utils import run_bass_kernel_spmd

F32 = mybir.dt.float32
F32R = mybir.dt.float32r
BF16 = mybir.dt.bfloat16
AF = mybir.ActivationFunctionType

B, N, D, C, K16 = 8, 4096, 128, 256, 16
QCOL = 1024          # psum quarter width (2 banks)
CHUNK = 512          # L1 max8 chunk width
LAM = 8.0e6          # penalty scale: exp arg gets -LAM * relu(t - pd)


def build_nc(n=N, pen_on_dve_tiles=0, chunk=CHUNK, recompute_pd=False, PENB=3, UNB=2, UNTB=2, OZB=3, spill_pd=False):
    """Build the single-core Bass program (same program runs SPMD on 8 cores)."""
    nc = bacc.Bacc("TRN2", target_bir_lowering=False, debug=False, num_devices=B)
    CHUNK_ = chunk
    ntiles = n // 128
    nquarters = n // QCOL if n >= QCOL else 1
    qcol = min(QCOL, n)

    at_d = nc.dram_tensor("at", [21, n], BF16, kind="ExternalInput").ap()
    bt_d = nc.dram_tensor("bt", [21, n], BF16, kind="ExternalInput").ap()
    ct_d = nc.dram_tensor("ct", [D, n], BF16, kind="ExternalInput").ap()
    ut_d = nc.dram_tensor("ut", [D, n], BF16, kind="ExternalInput").ap()
    ft_d = nc.dram_tensor("ft", [128, ntiles, C], BF16, kind="ExternalInput").ap()
    idn_d = nc.dram_tensor("idn", [128, 128], BF16, kind="ExternalInput").ap()
    idp_d = nc.dram_tensor("idp", [128, 128], BF16, kind="ExternalInput").ap()
    outt_d = nc.dram_tensor("outt", [C, n], F32, kind="ExternalOutput").ap()
    z_d = nc.dram_tensor("z", [n, 1], F32, kind="ExternalOutput").ap()

    with tile.TileContext(nc) as tc:
        with (
            tc.tile_pool(name="const", bufs=1) as constp,
            tc.tile_pool(name="pen", bufs=PENB) as penp,
            tc.tile_pool(name="un", bufs=UNB) as unp,
            tc.tile_pool(name="unt", bufs=UNTB) as untp,
            tc.tile_pool(name="sel", bufs=3) as selp,
            tc.tile_pool(name="oz", bufs=OZB) as ozp,
            tc.tile_pool(name="ps", bufs=4, space="PSUM") as psp,
        ):
            # persistent SBUF residents
            at_s = constp.tile([21, n], BF16, tag="at")
            nc.sync.dma_start(at_s[:], at_d[:])
            bt_s = constp.tile([21, n], BF16, tag="bt")
            nc.sync.dma_start(bt_s[:], bt_d[:])
            ct_s = constp.tile([D, n], BF16, tag="ct")
            nc.sync.dma_start(ct_s[:], ct_d[:])
            ut_s = constp.tile([D, n], BF16, tag="ut")
            nc.sync.dma_start(ut_s[:], ut_d[:])
            ft_s = constp.tile([128, ntiles, C], BF16, tag="ft")
            nc.sync.dma_start(ft_s[:], ft_d[:])
            idn_s = constp.tile([128, 128], BF16, tag="idn")
            nc.sync.dma_start(idn_s[:], idn_d[:])
            idp_s = constp.tile([128, 128], BF16, tag="idp")
            nc.sync.dma_start(idp_s[:], idp_d[:])
            obuf = constp.tile([128, 2, n], F32, tag="obuf")
            zbuf = constp.tile([128, ntiles], F32, tag="zbuf")

            atr = at_s[:]
            btr = bt_s[:]
            ctr = ct_s[:]
            utr = ut_s[:]

            for i in range(ntiles):
                i0 = i * 128
                # ---- phase A: distances + candidate extraction ----
                pd_q = []
                spill = None
                if spill_pd:
                    spill = penp.tile([128, n], F32, tag="spill")
                cand = selp.tile([128, (n // CHUNK_) * 8], F32, tag="cand")
                for q in range(nquarters):
                    qoff = q * qcol
                    pq = psp.tile([128, qcol], F32, tag="ps")
                    pd_q.append(pq)
                    for j in range(qcol // 512):
                        nc.tensor.matmul(
                            pq[:, j * 512 : (j + 1) * 512],
                            atr[:, i0 : i0 + 128],
                            btr[:, qoff + j * 512 : qoff + (j + 1) * 512],
                            start=True,
                            stop=True,
                        )
                    if spill_pd:
                        nc.scalar.copy(spill[:, qoff : qoff + qcol], pq[:])
                        src_q = spill[:, qoff : qoff + qcol]
                    else:
                        src_q = pq[:]
                    for cc in range(qcol // CHUNK_):
                        gidx = (q * (qcol // CHUNK_) + cc) * 8
                        nc.vector.max(
                            cand[:, gidx : gidx + 8],
                            src_q[:, cc * CHUNK_ : (cc + 1) * CHUNK_],
                        )
                # ---- L2: 16th largest candidate ----
                v8 = selp.tile([128, 8], F32, tag="v8")
                nc.vector.max(v8[:], cand[:])
                cand2 = selp.tile([128, (n // CHUNK_) * 8], F32, tag="cand2")
                nc.vector.match_replace(cand2[:], v8[:], cand[:], -1e30)
                v8b = selp.tile([128, 8], F32, tag="v8b")
                nc.vector.max(v8b[:], cand2[:])
                t_ap = v8b[:, 7:8]

                # ---- phase B/C: penalty, scores, exp, transpose ----
                un = unp.tile([128, n], BF16, tag="un")
                unt = untp.tile([128, n // 128, 128], BF16, tag="unt")
                zqs = []
                for q in range(nquarters):
                    qoff = q * qcol
                    if recompute_pd:
                        pq2 = psp.tile([128, qcol], F32, tag="ps")
                        for j in range(qcol // 512):
                            nc.tensor.matmul(
                                pq2[:, j * 512 : (j + 1) * 512],
                                atr[:, i0 : i0 + 128],
                                btr[:, qoff + j * 512 : qoff + (j + 1) * 512],
                                start=True,
                                stop=True,
                            )
                        pd_q[q] = pq2
                    pen = penp.tile([128, qcol], BF16, tag="pen")
                    on_dve = i < pen_on_dve_tiles
                    if on_dve:
                        # pen = min(pd - t, 0) <= 0, injected with +lam identity
                        nc.vector.tensor_scalar(
                            pen[:], pd_q[q][:], t_ap, 0.0,
                            mybir.AluOpType.subtract,
                            mybir.AluOpType.min,
                        )
                    else:
                        # pen = relu(t - pd) >= 0, injected with -lam identity
                        nc.scalar.activation(
                            pen[:], pd_q[q][:], AF.Relu, bias=t_ap, scale=-1.0
                        )
                    sq = psp.tile([128, qcol], F32, tag="ps")
                    for j in range(qcol // 512):
                        nc.tensor.matmul(
                            sq[:, j * 512 : (j + 1) * 512],
                            utr[:, i0 : i0 + 128],
                            ctr[:, qoff + j * 512 : qoff + (j + 1) * 512],
                            start=True,
                            stop=False,
                        )
                    for j in range(qcol // 512):
                        nc.tensor.matmul(
                            sq[:, j * 512 : (j + 1) * 512],
                            (idp_s if on_dve else idn_s)[:],
                            pen[:, j * 512 : (j + 1) * 512],
                            start=False,
                            stop=True,
                        )
                    zq = ozp.tile([128, 1], F32, tag="zq%d" % q)
                    zqs.append(zq)
                    nc.scalar.activation(
                        un[:, qoff : qoff + qcol], sq[:], AF.Exp, accum_out=zq[:]
                    )
                    nc.sync.dma_start_transpose(
                        unt[:, qoff // 128 : (qoff + qcol) // 128, :],
                        un[:, qoff : qoff + qcol],
                    )
                # ---- phase D: output matmul ----
                outt_big = psp.tile([128, qcol], F32, tag="ps")
                nch = n // 128
                for ch in range(nch):
                    for half in range(2):
                        hoff = half * 512  # separate psum banks per half
                        nc.tensor.matmul(
                            outt_big[:, hoff : hoff + 128],
                            ft_s[:, ch, half * 128 : (half + 1) * 128],
                            unt[:, ch, :],
                            start=(ch == 0),
                            stop=(ch == nch - 1),
                        )
                # ---- phase E: Z sum + copies into batched buffers ----
                zslot = zbuf[:, i : i + 1]
                if nquarters == 1:
                    nc.vector.tensor_copy(zslot, zqs[0][:])
                elif nquarters == 2:
                    nc.vector.tensor_add(zslot, zqs[0][:], zqs[1][:])
                else:
                    z01 = ozp.tile([128, 1], F32, tag="z01")
                    nc.vector.tensor_add(z01[:], zqs[0][:], zqs[1][:])
                    z23 = ozp.tile([128, 1], F32, tag="z23")
                    nc.vector.tensor_add(z23[:], zqs[2][:], zqs[3][:])
                    nc.vector.tensor_add(zslot, z01[:], z23[:])
                for half in range(2):
                    nc.vector.tensor_copy(
                        obuf[:, half, i0 : i0 + 128],
                        outt_big[:, half * 512 : half * 512 + 128],
                    )
            # ---- final batched DMAs ----
            for half in range(2):
                nc.sync.dma_start(
                    outt_d[half * 128 : (half + 1) * 128, :], obuf[:, half, :]
                )
            z_view = z_d.rearrange("(i p) one -> p (i one)", p=128)
            nc.sync.dma_start(z_view, zbuf[:])
    nc.compile()
    return nc


def host_prep(fp4_xyz, fp4_features, concatenate_features, Wq, bq, Wk, bk, n=N):
    """Per-core input dicts. Index b = core = batch element."""
    beta = 1.0 / np.sqrt(np.float32(D))
    Msm = (Wk.T @ Wq).astype(np.float32)
    vq = (Wk.T @ bq).astype(np.float32)
    idn = (-LAM * np.eye(128, dtype=np.float32)).astype(ml_dtypes.bfloat16)
    idp = (LAM * np.eye(128, dtype=np.float32)).astype(ml_dtypes.bfloat16)
    ntiles = n // 128
    maps = []
    bf = ml_dtypes.bfloat16
    for b in range(fp4_xyz.shape[0]):
        x = fp4_xyz[b].astype(np.float32)          # [N, 3]
        c = concatenate_features[b].astype(np.float32)  # [N, D]
        xx = (x * x).sum(1)                         # [N]
        # 3-way-split bf16 distance matmul, K=21 (fp32-grade precision):
        #   pd[n,m] = 2 x_n.x_m - xx_m, x = xh+xm+xl (bf16 splits),
        #   keeping products hh,hm,hl,mh,mm,lh and xx as 3 bf16 rows.
        xh = x.T.astype(bf).astype(np.float32)      # [3, N]
        xm = (x.T - xh).astype(bf).astype(np.float32)
        xl = (x.T - xh - xm).astype(bf).astype(np.float32)
        xxh = xx.astype(bf).astype(np.float32)
        xxm = (xx - xxh).astype(bf).astype(np.float32)
        xxl = (xx - xxh - xxm).astype(bf).astype(np.float32)
        ones = np.ones((1, n), np.float32)
        at = np.concatenate(
            [xh, xh, xh, xm, xm, xl, ones, ones, ones], 0
        )  # [21, N]
        bt = np.concatenate(
            [2 * xh, 2 * xm, 2 * xl, 2 * xh, 2 * xm, 2 * xh,
             -xxh[None], -xxm[None], -xxl[None]], 0
        )  # [21, N]
        ct = np.ascontiguousarray(c.T)              # [D, N]
        ut = (beta * (Msm @ ct + vq[:, None])).astype(np.float32)   # [D, N]
        ft = fp4_features[b].T.astype(np.float32)   # [N, C]
        ft = np.ascontiguousarray(
            ft.reshape(ntiles, 128, C).transpose(1, 0, 2)
        ).astype(ml_dtypes.bfloat16)                # [128, ntiles, C]
        maps.append(
            {
                "at": at.astype(bf),
                "bt": bt.astype(bf),
                "ct": ct.astype(bf),
                "ut": ut.astype(bf),
                "ft": ft,
                "idn": idn,
                "idp": idp,
            }
        )
    return maps


_NC_CACHE = {}


def kernel(**inputs):
    key = "full"
    if key not in _NC_CACHE:
        _NC_CACHE[key] = build_nc(recompute_pd=True, pen_on_dve_tiles=1 << 30)
    nc = _NC_CACHE[key]
    maps = host_prep(**inputs)
    core_ids = list(range(B))
    res = run_bass_kernel_spmd(nc, maps, core_ids)
    outs = []
    for b in range(B):
        outt = res.results[b]["outt"].astype(np.float32)   # [C, N] unnormalized
        z = res.results[b]["z"].astype(np.float32)[:, 0]   # [N]
        outs.append(outt / z[None, :])
    return np.stack(outs, 0).astype(np.float32)
